# revision 2
# baseline (speedup 1.0000x reference)
"""Multi-head causal attention with RoPE on 8 TRN2 NeuronCores — v2.

Sharding: 4-way data parallel on batch x 2-way tensor parallel on heads
(core c -> batch c//2, head-group c%2 of 8 heads).  No on-device
collectives: the two head-group partials per batch are summed on the host.

v2 vs baseline: q/k are computed DIRECTLY in [e, s] layout (weights
stationary on the PE, x streaming), so the rope->DRAM->XBAR-transpose
chain is gone.  RoPE in [e, s] uses host-permuted weight columns (per
head: even components then odd components), a cos table multiply, a
+/-sin table multiply, a 32-partition-block swap via 4 tiny SBUF->SBUF
DMAs, and one add:  out = P*cct + swap32(P*sst') with sst' carrying the
sign pattern [+sin, -sin] per half.  Inputs are loaded as per-(group, d)
tiles so the first matmul starts ~1us in instead of waiting for a 4MB
monolithic x load.  Emission: per 512-token group g: q/k strips, v
tiles, then attention burst g (same flash structure as baseline), with
the o-projection of group g-1 after burst g.
"""

import sys

if "/opt/trn_rl_repo" not in sys.path:
    sys.path.insert(0, "/opt/trn_rl_repo")

import numpy as np
import ml_dtypes

import concourse.bass as bass
import concourse.mybir as mybir
from concourse.bass_utils import run_bass_kernel_spmd
from concourse.tile import TileContext
from concourse.vector_clock import ScopedClock
from concourse import tile as tile_mod

bf16 = ml_dtypes.bfloat16
F32 = mybir.dt.float32
BF16 = mybir.dt.bfloat16

B, S, D = 4, 2048, 1024
H, DH = 16, 64           # total heads, head dim
HC = 8                   # heads per core
THETA = 10000.0
N_CORES = 8

# ----------------------------------------------------------------------------
# neuronxcc sync-wait-limit workarounds (this walrus build rejects >2 waits
# per instruction, and >1 on DMA pseudo-instructions).
# ----------------------------------------------------------------------------
_counter = [0]


def _patched_drain_and_barrier(self, tick_clock, wait_clock):
    nc = self.nc
    probe = nc.sync.nop(nofuse=True, hint="tail_drain_probe")
    wait_clock.add_sem_waits(probe.ins, ScopedClock({None: tick_clock.global_clock}))
    waits = []
    if probe.ins.sync_info and probe.ins.sync_info.on_wait:
        waits = list(probe.ins.sync_info.on_wait)
    if len(waits) > 1:
        probe.ins.sync_info.on_wait = waits[:1]
        for w in waits[1:]:
            nop = nc.sync.nop(nofuse=True, hint="tail_drain_split")
            si = nop.ins.sync_info
            if si is None:
                nop.ins.sync_info = mybir.SyncInfo(on_wait=[w], on_update=[])
            else:
                si.on_wait = [w]
    nc.sync.drain()
    nc.all_engine_barrier()
    assert self.sems is not None
    popped = nc._tile_sem_poison_stack.pop()
    assert popped is self._sem_poison
    nc.clear_and_free_semaphores(list(self.sems.allocated().values()))
    nc.all_engine_barrier()


tile_mod.TileContext._drain_and_barrier = _patched_drain_and_barrier


def _wait_budget(inst):
    return 1


def split_excess_waits(nc):
    for fn in nc.m.functions:
        for bb in fn.blocks:
            new_list = []
            for inst in bb.instructions:
                si = getattr(inst, "sync_info", None)
                waits = list(si.on_wait) if (si is not None and si.on_wait) else []
                budget = _wait_budget(inst)
                if len(waits) > budget:
                    extra = waits[:-budget] if budget > 0 else waits
                    for i in range(0, len(extra), 1):
                        chunk = extra[i : i + 1]
                        _counter[0] += 1
                        nop = mybir.InstNoOp(
                            name=f"I-waitsplit-{_counter[0]}", ins=[], outs=[]
                        )
                        nop.engine = inst.engine
                        nop.sync_info = mybir.SyncInfo(on_wait=chunk, on_update=[])
                        new_list.append(nop)
                    si.on_wait = waits[-budget:] if budget > 0 else []
                new_list.append(inst)
            bb.instructions[:] = new_list


# ----------------------------------------------------------------------------
# Device graph
# ----------------------------------------------------------------------------
def build_nc():
    nc = bass.Bass("TRN2", target_bir_lowering=False, debug=False,
                   num_devices=N_CORES)

    xt_ext = nc.declare_dram_parameter("xt", [D, S], BF16, isOutput=False)
    wq_ext = nc.declare_dram_parameter("wq", [D, 512], BF16, isOutput=False)
    wk_ext = nc.declare_dram_parameter("wk", [D, 512], BF16, isOutput=False)
    wv_ext = nc.declare_dram_parameter("wv", [D, 512], BF16, isOutput=False)
    wo_ext = nc.declare_dram_parameter("wo", [128, 4, D], BF16, isOutput=False)
    cc_ext = nc.declare_dram_parameter("cc", [128, S], BF16, isOutput=False)
    ss_ext = nc.declare_dram_parameter("ss", [128, S], BF16, isOutput=False)
    mk_ext = nc.declare_dram_parameter("mk", [128, 4, 2, 512], BF16, isOutput=False)
    out_ext = nc.declare_dram_parameter("out", [D, S], F32, isOutput=True)

    Exp = mybir.ActivationFunctionType.Exp
    Ln = mybir.ActivationFunctionType.Ln
    mult = mybir.AluOpType.mult
    add = mybir.AluOpType.add

    with TileContext(nc) as tc:
        with (
            tc.tile_pool(name="persist", bufs=1) as pers,
            tc.tile_pool(name="ring", bufs=2) as ring,
            tc.tile_pool(name="dbounce", bufs=8, space="DRAM") as dbounce,
            tc.tile_pool(name="rope", bufs=4) as rope,
            tc.tile_pool(name="est", bufs=8) as est,
            tc.tile_pool(name="epi", bufs=4) as epi,
            tc.tile_pool(name="ps1", bufs=2, space="PSUM") as ps1,
            tc.tile_pool(name="psS", bufs=2, space="PSUM") as psS,
            tc.tile_pool(name="psP", bufs=2, space="PSUM") as psP,
        ):
            # --- persistent SBUF ---
            vsb = [pers.tile([128, HC, 65], BF16, tag=f"vsb{s}", name=f"vsb{s}")
                   for s in range(16)]
            ot = [[pers.tile([128, 512], BF16, tag=f"ot{p}_{g}", name=f"ot{p}_{g}")
                   for g in range(4)] for p in range(4)]
            masks = pers.tile([128, 4, 2, 512], BF16, tag="masks")
            wo_t = pers.tile([128, 4, D], BF16, tag="wo")
            kt_t = [[pers.tile([128, 512], BF16, tag=f"kt{p}_{g}",
                               name=f"kt{p}_{g}") for g in range(4)]
                    for p in range(4)]
            wq_d = [pers.tile([128, 512], BF16, tag=f"wq{d}", name=f"wq{d}")
                    for d in range(8)]
            wk_d = [pers.tile([128, 512], BF16, tag=f"wk{d}", name=f"wk{d}")
                    for d in range(8)]
            wv_d = [pers.tile([128, 512], BF16, tag=f"wv{d}", name=f"wv{d}")
                    for d in range(8)]
            ones_b = pers.tile([128, 64], BF16, tag="ones_b")
            nc.vector.memset(ones_b[:], 1.0)
            for s in range(16):
                nc.vector.memset(vsb[s][:, :, 64:65], 1.0)

            # ring-buffered per-group tiles (bufs=2: group g reuses g-2's slot)
            xt_cur = {}    # d -> tile for the group being emitted
            cs_cur = [None, None]
            qt_cur = {}    # p -> tile for current group's roped q strip

            def emit_group_loads(g):
                gcol = slice(g * 512, (g + 1) * 512)
                for d in range(8):
                    t = ring.tile([128, 512], BF16, tag=f"xt{d}",
                                  name=f"xt{d}_g{g}")
                    nc.sync.dma_start(t[:], xt_ext[d * 128:(d + 1) * 128, gcol])
                    xt_cur[d] = t
                cc_g = ring.tile([128, 512], BF16, tag="cct", name=f"cct{g}")
                ss_g = ring.tile([128, 512], BF16, tag="sst", name=f"sst{g}")
                nc.sync.dma_start(cc_g[:], cc_ext[:, gcol])
                nc.sync.dma_start(ss_g[:], ss_ext[:, gcol])
                cs_cur[0], cs_cur[1] = cc_g, ss_g

            pend_add = []   # deferred rope adds: (dst, t0, t1s)

            def flush_adds(keep=0):
                while len(pend_add) > keep:
                    dst, t0, t1s = pend_add.pop(0)
                    nc.vector.tensor_tensor(dst[:], t0[:], t1s[:], add)

            def emit_qk_strip(nm, w_d, dst, p, g, xt_g, cc_g, ss_g):
                # The rope add is deferred one strip so the DVE stream never
                # sits waiting on the 32-partition swap DMAs mid-queue.
                pcol = slice(p * 128, (p + 1) * 128)
                ps = ps1.tile([128, 512], F32, tag="psq", name=f"ps{nm}{p}g{g}")
                for d in range(8):
                    nc.tensor.matmul(ps[:], lhsT=w_d[d][:, pcol],
                                     rhs=xt_g[d][:],
                                     start=(d == 0), stop=(d == 7))
                qsb = rope.tile([128, 512], BF16, tag="qsb")
                nc.scalar.copy(qsb[:], ps[:])
                t0 = rope.tile([128, 512], BF16, tag="t0")
                t1 = rope.tile([128, 512], BF16, tag="t1")
                t1s = rope.tile([128, 512], BF16, tag="t1s")
                nc.vector.tensor_tensor(t0[:], qsb[:], cc_g[:], mult)
                nc.vector.tensor_tensor(t1[:], qsb[:], ss_g[:], mult)
                for h in range(4):
                    a, b = h * 32, (h ^ 1) * 32
                    nc.sync.dma_start(t1s[a:a + 32, :], t1[b:b + 32, :])
                flush_adds(keep=1)
                pend_add.append((dst, t0, t1s))

            qt_hist = {}   # (p, g) -> tile

            def gen_qkv(g):
                xt_g = dict(xt_cur)      # snapshot: loads for g already emitted
                cc_g, ss_g = cs_cur
                for p in range(4):
                    qt = ring.tile([128, 512], BF16, tag=f"qt{p}",
                                   name=f"qt{p}_g{g}")
                    qt_hist[(p, g)] = qt
                    emit_qk_strip("q", wq_d, qt, p, g, xt_g, cc_g, ss_g)
                    yield
                    emit_qk_strip("k", wk_d, kt_t[p][g], p, g, xt_g, cc_g, ss_g)
                    yield
                for st in range(4):
                    s = 4 * g + st
                    scol = slice(st * 128, (st + 1) * 128)
                    psv = ps1.tile([128, 512], F32, tag="psq", name=f"psv{s}")
                    for d in range(8):
                        nc.tensor.matmul(psv[:], lhsT=xt_g[d][:, scol],
                                         rhs=wv_d[d][:],
                                         start=(d == 0), stop=(d == 7))
                    nc.vector.tensor_copy(
                        out=vsb[s][:, :, 0:64],
                        in_=psv[:].rearrange("p (h c) -> p h c", h=HC))
                    flush_adds(keep=0)
                    yield

            def _gen_units(gen):
                # wrap a generator into a list of single-step closures
                def step(it):
                    def f():
                        next(it, None)
                    return f
                it = iter(gen)
                return it

            def qkv_units(g):
                it = iter(gen_qkv(g))
                return [(lambda it=it: next(it, None)) for _ in range(12)]

            def oproj_units(b):
                it = iter(gen_oproj(b))
                return [(lambda it=it: next(it, None)) for _ in range(8)]

            def burst_units(j):
                it = iter(gen_burst(j))
                return [(lambda it=it: next(it, None))
                        for _ in range(16 * (j + 1))]

            def gen_burst(j):
                n_k = 4 * j + 4
                jcol = slice(j * 512, (j + 1) * 512)
                steps = [(p, i) for p in range(4) for i in range(n_k)]
                pss_t = {}

                def emit_scores(t):
                    p, i = steps[t]
                    qoff = max(0, (i - 4 * j) * 128)
                    pss = psS.tile([128, 2, 512], F32, tag="pss",
                                   name=f"pss{p}_{j}_{i}")
                    for hh in range(2):
                        prow = slice(hh * 64, (hh + 1) * 64)
                        nc.tensor.matmul(
                            pss[:, hh, qoff:512],
                            lhsT=kt_t[p][i // 4][prow,
                                                 (i % 4) * 128:(i % 4 + 1) * 128],
                            rhs=qt_hist[(p, j)][prow, qoff:512],
                            start=True, stop=True)
                    pss_t[t] = pss

                emitted = 0
                pv_cur = None
                for t in range(len(steps)):
                    while emitted < min(t + 3, len(steps)):
                        emit_scores(emitted)
                        emitted += 1
                    p, i = steps[t]
                    if i == 0:
                        pv_cur = [psP.tile([65, 512], F32, tag="pv",
                                           name=f"pv{p}_{j}_{k}")
                                  for k in range(2)]
                    pss_cur = pss_t.pop(t)
                    qoff = max(0, (i - 4 * j) * 128)
                    nw = 512 - qoff
                    e_t = est.tile([128, 2, 512], BF16, tag="est",
                                   name=f"est{p}_{j}_{i}")
                    nc.scalar.activation(e_t[:, :, qoff:512],
                                         pss_cur[:, :, qoff:512],
                                         Exp, scale=0.125)
                    if i >= 4 * j:
                        nc.vector.tensor_tensor(
                            e_t[:, :, qoff:512], e_t[:, :, qoff:512],
                            masks[:, 0, :, 0:nw], mult)
                    for hh in range(2):
                        nc.tensor.matmul(
                            pv_cur[hh][:, qoff:512],
                            lhsT=vsb[i][:, 2 * p + hh, :],
                            rhs=e_t[:, hh, qoff:512],
                            start=(i == 0), stop=(i == n_k - 1))
                    if i == n_k - 1:
                        for hh in range(2):
                            # engine-only normalization: den row -> PE
                            # broadcast over 64 partitions -> 1/den =
                            # exp(-ln(den)) on ACT -> scale.  No DMA hops.
                            # pv is copied to SBUF first so its PSUM bank
                            # frees for the next head-pair immediately.
                            pv = pv_cur[hh]
                            pvs = epi.tile([65, 512], BF16, tag="pvs",
                                           name=f"pvs{p}_{j}_{hh}")
                            nc.vector.tensor_copy(out=pvs[:], in_=pv[:])
                            bc = ps1.tile([64, 512], F32, tag="psq",
                                          name=f"bc{p}_{j}_{hh}")
                            nc.tensor.matmul(bc[:], lhsT=ones_b[64:65, :],
                                             rhs=pvs[64:65, :],
                                             start=True, stop=True)
                            lg = epi.tile([64, 512], F32, tag="lg")
                            nc.scalar.activation(lg[:], bc[:], Ln)
                            rb = epi.tile([64, 512], F32, tag="rb")
                            nc.scalar.activation(rb[:], lg[:], Exp, scale=-1.0)
                            if hh == 0:
                                nc.vector.tensor_tensor(
                                    ot[p][j][0:64, :], pvs[0:64, :], rb[:], mult)
                            else:
                                stg = epi.tile([64, 512], BF16, tag="stg")
                                nc.vector.tensor_tensor(
                                    stg[:], pvs[0:64, :], rb[:], mult)
                                nc.sync.dma_start(ot[p][j][64:128, :], stg[:])
                    yield

            def gen_oproj(sb_, trailing=False):
                scol = slice(sb_ * 512, (sb_ + 1) * 512)
                for mt in range(8):
                    mcol = slice(mt * 128, (mt + 1) * 128)
                    if trailing:
                        pso = psS.tile([128, 512], F32, tag="pss",
                                       name=f"pso{mt}_{sb_}")
                    else:
                        pso = ps1.tile([128, 512], F32, tag="psq",
                                       name=f"pso{mt}_{sb_}")
                    for p in range(4):
                        nc.tensor.matmul(pso[:], lhsT=wo_t[:, p, mcol],
                                         rhs=ot[p][sb_][:],
                                         start=(p == 0), stop=(p == 3))
                    og = est.tile([128, 512], F32, tag="est",
                                  name=f"og{mt}_{sb_}")
                    nc.vector.tensor_copy(out=og[:], in_=pso[:])
                    nc.sync.dma_start(out_ext[mcol, scol], og[:])
                    yield

            # ---- emission schedule ----
            # Load order feeds the first strips earliest: x-group0 + wq
            # interleaved, then wk, rope tables, wv, then masks/wo.
            gcol0 = slice(0, 512)
            xt0 = {}
            for d in range(8):
                drow = slice(d * 128, (d + 1) * 128)
                t = ring.tile([128, 512], BF16, tag=f"xt{d}", name=f"xt{d}_g0")
                nc.sync.dma_start(t[:], xt_ext[drow, gcol0])
                xt0[d] = t
                nc.sync.dma_start(wq_d[d][:], wq_ext[drow, :])
            for d in range(8):
                nc.sync.dma_start(wk_d[d][:], wk_ext[d * 128:(d + 1) * 128, :])
            cc_0 = ring.tile([128, 512], BF16, tag="cct", name="cct0")
            ss_0 = ring.tile([128, 512], BF16, tag="sst", name="sst0")
            nc.sync.dma_start(cc_0[:], cc_ext[:, gcol0])
            nc.sync.dma_start(ss_0[:], ss_ext[:, gcol0])
            for d in range(8):
                nc.sync.dma_start(wv_d[d][:], wv_ext[d * 128:(d + 1) * 128, :])
            nc.sync.dma_start(masks[:], mk_ext[:])
            nc.sync.dma_start(wo_t[:], wo_ext[:])
            xt_cur.update(xt0)
            cs_cur[0], cs_cur[1] = cc_0, ss_0

            def run_all(gen):
                for _ in gen:
                    pass

            def weave(main, fillers):
                # interleave filler units evenly across the main generator's
                # yield points (PE gets ACT-independent work while the
                # scalar engine digests the burst's exp backlog)
                fill = []
                for f in fillers:
                    fill.extend(f)
                n_main = len(main)
                k = 0
                for t, unit in enumerate(main):
                    unit()
                    want = min(len(fill), (t + 1) * len(fill) // n_main)
                    while k < want:
                        fill[k]()
                        k += 1
                while k < len(fill):
                    fill[k]()
                    k += 1

            run_all(gen_qkv(0))
            emit_group_loads(1)
            weave(burst_units(0), [qkv_units(1)])
            emit_group_loads(2)
            weave(burst_units(1), [qkv_units(2)])
            emit_group_loads(3)
            weave(burst_units(2), [qkv_units(3), oproj_units(0)])
            weave(burst_units(3), [oproj_units(1), oproj_units(2)])
            run_all(gen_oproj(3, trailing=True))
            del xt0

    split_excess_waits(nc)
    return nc


# ----------------------------------------------------------------------------
# Host-side input prep / unshard
# ----------------------------------------------------------------------------
def _rope_tables(token_positions):
    inv = THETA ** (-np.arange(0, DH // 2, dtype=np.float32) * 2.0 / DH)  # [32]
    ang = token_positions.astype(np.float32)[None, :] * inv[:, None]     # [32, S]
    cos, sin = np.cos(ang), np.sin(ang)
    # cct rows (p % 32 indexes the frequency, repeating every 32 partitions);
    # sst carries sign + for rows (p % 64) < 32, - otherwise.
    cct = np.tile(cos, (4, 1))                                           # [128, S]
    sst = np.tile(np.concatenate([sin, -sin], axis=0), (2, 1))           # [128, S]
    return cct.astype(bf16), sst.astype(bf16)


def _perm():
    p = []
    for h in range(HC):
        base = h * DH
        p.extend(base + np.arange(0, DH, 2))
        p.extend(base + np.arange(1, DH, 2))
    return np.asarray(p)


def prep_in_maps(x, token_positions, q_w, k_w, v_w, o_w):
    x = np.asarray(x); token_positions = np.asarray(token_positions)
    q_w = np.asarray(q_w); k_w = np.asarray(k_w)
    v_w = np.asarray(v_w); o_w = np.asarray(o_w)

    cct, sst = _rope_tables(token_positions)
    perm = _perm()
    mk1 = (np.arange(512)[None, None, :] >=
           (np.arange(128)[None, :, None] + 128 * np.arange(4)[:, None, None])
           ).astype(bf16)
    mk = np.concatenate([mk1, mk1], axis=2)              # [4, 128, 1024]
    # device layout [k 128, m 4, h 2, q 512] so the load is one dense
    # 128-descriptor DMA instead of a ~1k-descriptor gather
    mk = np.ascontiguousarray(
        mk.reshape(4, 128, 2, 512).transpose(1, 0, 2, 3))

    in_maps = []
    for c in range(N_CORES):
        b, hg = c // 2, c % 2
        esl = slice(hg * 512, (hg + 1) * 512)
        wq = q_w[esl, :][perm, :].T.astype(bf16)      # [D, 512]
        wk = k_w[esl, :][perm, :].T.astype(bf16)
        wv = v_w[esl, :].T.astype(bf16)
        wo = o_w[:, esl].T.astype(bf16)               # [512, D]
        wo = np.ascontiguousarray(
            wo.reshape(4, 128, D).transpose(1, 0, 2))  # [128, 4, D]
        in_maps.append({
            "xt": np.ascontiguousarray(x[b].T).astype(bf16),
            "wq": np.ascontiguousarray(wq), "wk": np.ascontiguousarray(wk),
            "wv": np.ascontiguousarray(wv), "wo": np.ascontiguousarray(wo),
            "cc": cct, "ss": sst, "mk": mk,
        })
    return in_maps


def unshard(results):
    out = np.empty((B, S, D), dtype=np.float32)
    for b in range(B):
        out[b] = (results[2 * b]["out"] + results[2 * b + 1]["out"]).T
    return out


_nc_cache = [None]


def kernel(x, token_positions, q_w, k_w, v_w, o_w):
    if _nc_cache[0] is None:
        _nc_cache[0] = build_nc()
    nc = _nc_cache[0]
    in_maps = prep_in_maps(x, token_positions, q_w, k_w, v_w, o_w)
    res = run_bass_kernel_spmd(nc, in_maps, list(range(N_CORES)))
    return unshard(res.results)


if __name__ == "__main__":
    rng = np.random.default_rng(0)
    x = rng.standard_normal((B, S, D), dtype=np.float32)
    tp = np.arange(S, dtype=np.int32)
    sc = 1.0 / np.sqrt(D)
    ws = [rng.standard_normal((D, D), dtype=np.float32) * sc for _ in range(4)]
    out = kernel(x, tp, *ws)
    print("kernel ran, out shape", out.shape, "mean", float(np.abs(out).mean()))
